# revision 17
# baseline (speedup 1.0000x reference)
"""Trainium2 Bass kernel for nn_BondMessagePassing (D-MPNN style GNN).

Contract: kernel(**inputs) takes FULL unsharded inputs (as produced by the
reference's setup_inputs) and returns the FULL output [400000, 128] float32.

Math: the reference builds edges in exact reverse pairs (edge 2k+1 is the
reverse of edge 2k, rev_edge_index = i^1), which makes dst[rev] == src.
Consequently the two scatter-adds inside every message-passing iteration
cancel exactly, so m == 0 through the loop and the output reduces to

    h   = relu([x[src], edge_attr] @ W_i)
    m   = scatter_add(h, dst)
    out = relu([x, m] @ W_o)

(biases are all zero for the documented generator; nonzero biases or a
broken reverse-pair identity fall back to an exact numpy replication).

Sharding: nodes are range-partitioned across the 8 cores (50000 nodes
each); each core receives exactly the edges whose dst lands in its range
(sorted by dst) so the scatter-add is core-local and the output rows are a
contiguous slice -- no collectives.

v2 design (vs the 180us baseline):
 - Variable-boundary windows: each 512-node supertile is split into <=5
   node ranges chosen on the host so that EVERY core has <=128 edges per
   range (edges are sorted by dst, so a window's edges are a consecutive
   run; slot = 128*window + rank). This removes the overflow tile
   entirely: the scatter-add streams exactly 512 one-hot columns per
   supertile (down from 1024) and the one-hot DMA drops from 104KB to
   64KB per supertile.
 - zt and xct are fp8e3m4 (4 mantissa bits): halves the edge-payload DMA
   vs fp16 at ~1e-3 cost in relative error (1e-2 total, gate is 2e-2).
 - PSUM drains are spread over three engines: scalar does the h-relu,
   vector does the m-cast, gpsimd (Pool) does the out-relu.
 - The PE stream is software-pipelined across supertiles:
   [h0(g+1), out1(g+1), scatter(g), out2(g-1)] so the PE never sits
   behind a drain it just requested.
"""

import ml_dtypes
import numpy as np

# ---- problem constants (hardcoded per contract) ----
N = 400000
E = 400000
XD = 64        # node feature dim
EAD = 16       # edge feature dim
HID = 128
DEPTH = 5
NCORES = 8
NL = N // NCORES          # nodes per core
SUP = 512                 # nodes per supertile (one PSUM bank of fp32)
NSUP = (NL + SUP - 1) // SUP
NPAD = NSUP * SUP
P = 128
MAXW = 6                  # max windows per supertile supported by the program

F16 = np.float16
F32 = np.float32
F8E4 = ml_dtypes.float8_e4m3
F8E3 = ml_dtypes.float8_e3m4


def _check_fast_path_ok(src, dst, rev, x, edge_attr, b_i, b_h, b_o):
    """True iff the loop-cancellation identity holds, biases are zero, and
    fp8/fp16 is safe."""
    if src.shape != (E,) or dst.shape != (E,) or rev.shape != (E,):
        return False
    if np.any(b_i) or np.any(b_h) or np.any(b_o):
        return False
    if rev.min() < 0 or rev.max() >= E:
        return False
    seen = np.zeros(E, dtype=bool)
    seen[rev] = True
    if not seen.all():
        return False
    if not np.array_equal(dst[rev], src):
        return False
    if src.min() < 0 or src.max() >= N or dst.min() < 0 or dst.max() >= N:
        return False
    mx = float(np.abs(x).max(initial=0.0))
    mea = float(np.abs(edge_attr).max(initial=0.0))
    if max(mx, mea) > 14.0:       # fp8e3m4 max is 15.5
        return False
    return True


def _reference_fallback(x, edge_index, edge_attr, rev_edge_index,
                        W_i, b_i, W_h, b_h, W_o, b_o):
    def san(t):
        return np.nan_to_num(t, nan=0.0, posinf=1000.0, neginf=-1000.0)

    src, dst = edge_index[0], edge_index[1]
    h0 = np.maximum(
        np.concatenate([x[src], edge_attr], axis=1) @ W_i + b_i, 0.0
    ).astype(F32)
    h = h0
    for _ in range(1, DEPTH):
        m = np.zeros_like(h)
        np.add.at(m, dst, h)
        np.add.at(m, src, -h[rev_edge_index])
        m = san(m) @ W_h + b_h
        h = np.maximum(h0 + m, 0.0).astype(F32)
    m_final = np.zeros_like(h)
    np.add.at(m_final, dst, h)
    h_cat = np.concatenate([x, san(m_final)], axis=1)
    out = np.maximum(h_cat @ W_o + b_o, 0.0).astype(F32)
    return san(out)


def _chunk_schedule():
    # even-sized chunks (supertiles are processed in pairs)
    sched = []
    t0 = 0
    for g in (2, 4):
        sched.append((t0, g)); t0 += g
    while NSUP - t0 > 12:
        sched.append((t0, 12)); t0 += 12
    for g in (6, 4, 2):
        if NSUP - t0 >= g:
            sched.append((t0, g)); t0 += g
    while t0 < NSUP:
        sched.append((t0, 2)); t0 += 2
    assert sum(g for _, g in sched) == NSUP
    assert all(g % 2 == 0 for _, g in sched)
    return sched


_PROGRAM_CACHE = {}


def _build_program(win_key):
    """Build the SPMD Bass program.

    win_key: tuple over supertiles of tuples of (offset, width) windows,
    offsets local to the supertile, widths summing to 512. Each window has
    <=128 edges on every core.
    """
    import concourse.bacc as bacc
    import concourse.mybir as mybir
    import concourse.tile as tile

    wins = [list(ws) for ws in win_key]
    nwl = [len(ws) for ws in wins]
    wib = np.concatenate(([0], np.cumsum(nwl)))   # window-id base per T
    NWT = int(wib[-1])

    nc = bacc.Bacc("TRN2", target_bir_lowering=False, debug=False,
                   num_devices=NCORES)
    dt = mybir.dt

    zt = nc.dram_tensor("zt", [80, NWT * P], dt.float8e3,
                        kind="ExternalInput")
    s4d = nc.dram_tensor("s4d", [P, NSUP * SUP], dt.float8e4,
                         kind="ExternalInput")
    xct = nc.dram_tensor("xct", [XD, NPAD], dt.float8e3,
                         kind="ExternalInput")
    w_ih = nc.dram_tensor("w_ih", [80, HID], dt.float16, kind="ExternalInput")
    w_o1 = nc.dram_tensor("w_o1", [XD, HID], dt.float16, kind="ExternalInput")
    w_o2 = nc.dram_tensor("w_o2", [HID, HID], dt.float16, kind="ExternalInput")
    # output produced TRANSPOSED ([hidden, node]); host transposes back
    outT = nc.dram_tensor("outT", [HID, NPAD], dt.float16,
                          kind="ExternalOutput")

    RELU = mybir.ActivationFunctionType.Relu
    sched = _chunk_schedule()
    nchunks = len(sched)
    chunk_of = np.empty(NSUP, dtype=np.int64)
    for j, (TT, Gc) in enumerate(sched):
        chunk_of[TT:TT + Gc] = j

    with tile.TileContext(nc) as tc:
        with (
            tc.tile_pool(name="consts", bufs=1) as consts,
            tc.tile_pool(name="ztp", bufs=3) as ztp,
            tc.tile_pool(name="sp", bufs=3) as sp,
            tc.tile_pool(name="xctp", bufs=3) as xctp,
            tc.tile_pool(name="hp", bufs=2) as hp,
            tc.tile_pool(name="mp", bufs=2) as mp,
            tc.tile_pool(name="op", bufs=3) as op,
            tc.tile_pool(name="hps", bufs=1, space="PSUM") as hps,
            tc.tile_pool(name="mps", bufs=1, space="PSUM") as mps,
            tc.tile_pool(name="ops", bufs=1, space="PSUM") as ops,
        ):
            zt_c, s4_c, xct_c, o_buf = {}, {}, {}, {}

            def issue_chunk(j):
                TT, Gc = sched[j]
                zc0, zc1 = int(wib[TT]) * P, int(wib[TT + Gc]) * P
                zt_t = ztp.tile([80, zc1 - zc0], dt.float8e3, tag="ztc",
                                name=f"zt_{j}")
                nc.sync.dma_start(out=zt_t, in_=zt[:, zc0:zc1])
                zt_c[j] = zt_t
                s4_t = sp.tile([P, Gc * SUP], dt.float8e4, tag="s4c",
                               name=f"s4_{j}")
                nc.sync.dma_start(
                    out=s4_t, in_=s4d[:, TT * SUP:(TT + Gc) * SUP])
                s4_c[j] = s4_t
                xct_t = xctp.tile([XD, Gc * SUP], dt.float8e3, tag="xctc",
                                  name=f"xct_{j}")
                nc.sync.dma_start(
                    out=xct_t, in_=xct[:, TT * SUP:(TT + Gc) * SUP])
                xct_c[j] = xct_t

            issue_chunk(0)
            w_ih_t = consts.tile([80, HID], dt.float16)
            nc.scalar.dma_start(out=w_ih_t, in_=w_ih[:, :])
            w_o1_t = consts.tile([XD, HID], dt.float16)
            nc.scalar.dma_start(out=w_o1_t, in_=w_o1[:, :])
            w_o2_t = consts.tile([HID, HID], dt.float16)
            nc.scalar.dma_start(out=w_o2_t, in_=w_o2[:, :])

            issue_chunk(1)

            h_sb_t, m_t_t, o_ps_t = {}, {}, {}
            NPAIR = NSUP // 2

            for i in range(NPAIR + 2):
                # --- A/C: h0 matmuls + one paired h-relu for pair i ---
                pA = i
                if pA < NPAIR:
                    g0 = 2 * pA
                    j = int(chunk_of[g0])
                    if g0 == sched[j][0] and j + 2 < nchunks and \
                            (j + 2) not in zt_c:
                        issue_chunk(j + 2)
                    TT, Gc = sched[j]
                    h_ps = hps.tile([P, 10 * HID], mybir.dt.float32,
                                    tag="hps", name=f"hps_{pA}")
                    nwin = [nwl[g0], nwl[g0 + 1]]
                    for s in (0, 1):
                        g = g0 + s
                        zoff = (int(wib[g]) - int(wib[TT])) * P
                        for w in range(nwin[s]):
                            c0 = (s * 5 + w) * HID
                            nc.tensor.matmul(
                                h_ps[:, c0:c0 + HID],
                                zt_c[j][:, zoff + w * P:zoff + (w + 1) * P],
                                w_ih_t, start=True, stop=True,
                                skip_group_check=True)
                    # one relu for the whole pair (two only in the rare case
                    # of a <5-window first supertile, to avoid reading the
                    # uninitialized PSUM gap)
                    h_sb = hp.tile([P, 10 * HID], dt.float16,
                                   tag="hsb", name=f"hsb_{pA}")
                    h_sb_t[pA] = h_sb
                    if nwin[0] == 5:
                        hi_col = (5 + nwin[1]) * HID
                        nc.scalar.activation(h_sb[:, :hi_col],
                                             h_ps[:, :hi_col], RELU)
                    else:
                        nc.scalar.activation(
                            h_sb[:, :nwin[0] * HID],
                            h_ps[:, :nwin[0] * HID], RELU)
                        nc.scalar.activation(
                            h_sb[:, 5 * HID:(5 + nwin[1]) * HID],
                            h_ps[:, 5 * HID:(5 + nwin[1]) * HID], RELU)

                # --- F/G: out2 + out relu halves for pair i-2 ---
                pF = i - 2
                if 0 <= pF < NPAIR:
                    g0 = 2 * pF
                    j = int(chunk_of[g0])
                    TT, Gc = sched[j]
                    if j not in o_buf:
                        o_buf[j] = op.tile([P, Gc * SUP], dt.float16,
                                           tag="obuf", name=f"ob_{j}")
                    o_ps = o_ps_t.pop(pF)
                    for s in (0, 1):
                        nc.tensor.matmul(o_ps[:, s * SUP:(s + 1) * SUP],
                                         w_o2_t,
                                         m_t_t[pF][:, s * SUP:(s + 1) * SUP],
                                         start=False, stop=True,
                                         skip_group_check=True)
                    del m_t_t[pF]
                    gl = g0 - TT
                    ob = o_buf[j][:, gl * SUP:(gl + 2) * SUP]
                    if pF % 2 == 0:
                        nc.scalar.activation(ob, o_ps, RELU)
                    else:
                        nc.vector.tensor_scalar_max(ob, o_ps, 0.0)
                    if g0 + 2 == TT + Gc:
                        # chunk finished: ship it from the scalar DGE queue
                        nc.scalar.dma_start(
                            out=outT[:, TT * SUP:(TT + Gc) * SUP],
                            in_=o_buf[j])

                # --- D/E: scatter + paired m cast for pair i-1 ---
                pD = i - 1
                if 0 <= pD < NPAIR:
                    g0 = 2 * pD
                    j = int(chunk_of[g0])
                    TT, Gc = sched[j]
                    m_ps = mps.tile([P, 2 * SUP], mybir.dt.float32,
                                    tag="mps", name=f"mps_{pD}")
                    h_sb = h_sb_t.pop(pD)
                    for s in (0, 1):
                        g = g0 + s
                        gl = g - TT
                        for w, (off, width) in enumerate(wins[g]):
                            nc.tensor.matmul(
                                m_ps[:, s * SUP + off:s * SUP + off + width],
                                h_sb[:, (s * 5 + w) * HID:
                                     (s * 5 + w + 1) * HID],
                                s4_c[j][:, gl * SUP + off:
                                        gl * SUP + off + width],
                                start=True, stop=True, skip_group_check=True)
                    m_t = mp.tile([P, 2 * SUP], dt.float16, tag="mt",
                                  name=f"mt_{pD}")
                    m_t_t[pD] = m_t
                    nc.vector.tensor_copy(m_t, m_ps)

                # --- B: out1 for pair i (last: single-buffer o_ps reuse) ---
                if pA < NPAIR:
                    g0 = 2 * pA
                    j = int(chunk_of[g0])
                    TT, Gc = sched[j]
                    o_ps = ops.tile([P, 2 * SUP], mybir.dt.float32,
                                    tag="ops", name=f"ops_{pA}")
                    o_ps_t[pA] = o_ps
                    for s in (0, 1):
                        g = g0 + s
                        gl = g - TT
                        nc.tensor.matmul(o_ps[:, s * SUP:(s + 1) * SUP],
                                         w_o1_t,
                                         xct_c[j][:, gl * SUP:(gl + 1) * SUP],
                                         start=True, stop=False,
                                         skip_group_check=True)

    nc.compile()
    return nc


def kernel(**inputs):
    x = np.ascontiguousarray(np.asarray(inputs["x"]), dtype=F32)
    edge_index = np.asarray(inputs["edge_index"]).astype(np.int64)
    edge_attr = np.ascontiguousarray(np.asarray(inputs["edge_attr"]), dtype=F32)
    rev = np.asarray(inputs["rev_edge_index"]).astype(np.int64)
    W_i = np.asarray(inputs["W_i"], dtype=F32)
    b_i = np.asarray(inputs["b_i"], dtype=F32)
    W_h = np.asarray(inputs["W_h"], dtype=F32)
    b_h = np.asarray(inputs["b_h"], dtype=F32)
    W_o = np.asarray(inputs["W_o"], dtype=F32)
    b_o = np.asarray(inputs["b_o"], dtype=F32)

    src, dst = edge_index[0], edge_index[1]

    if not _check_fast_path_ok(src, dst, rev, x, edge_attr, b_i, b_h, b_o):
        return _reference_fallback(x, edge_index, edge_attr, rev,
                                   W_i, b_i, W_h, b_h, W_o, b_o)

    from concourse.bass_utils import run_bass_kernel_spmd

    # ---- host-side graph partition / sort (indices only) ----
    order = np.argsort(dst, kind="stable")
    dst_s = dst[order]
    core_starts = np.searchsorted(dst_s, np.arange(0, N + NL, NL))

    lds = []                       # per-core sorted local dst
    cums = np.zeros((NCORES, NPAD + 1), dtype=np.int64)
    for c in range(NCORES):
        e0, e1 = core_starts[c], core_starts[c + 1]
        ld = dst_s[e0:e1] - c * NL
        lds.append(ld)
        cnt = np.bincount(ld, minlength=NPAD)
        cums[c, 1:] = np.cumsum(cnt)

    # ---- joint greedy window boundaries (shared across all 8 cores) ----
    wins = []                      # per T: list of (offset, width)
    win_starts = []                # flat window start node ids (local)
    ok = True
    for T in range(NSUP):
        lo, hi = T * SUP, (T + 1) * SUP
        b = lo
        ws = []
        while b < hi:
            b2 = hi
            for c in range(NCORES):
                t = cums[c, b] + P
                bb = int(np.searchsorted(cums[c], t, side="right")) - 1
                b2 = min(b2, bb)
            if b2 <= b:
                ok = False
                break
            ws.append((b - lo, b2 - b))
            win_starts.append(b)
            b = b2
        if not ok or len(ws) > MAXW:
            ok = False
            break
        wins.append(tuple(ws))
    if not ok:
        return _reference_fallback(x, edge_index, edge_attr, rev,
                                   W_i, b_i, W_h, b_h, W_o, b_o)

    win_starts = np.asarray(win_starts, dtype=np.int64)
    NWT = len(win_starts)
    win_key = tuple(wins)

    # ---- shared constant tensors ----
    w_ih_np = np.ascontiguousarray(W_i).astype(F16)          # [80,128]
    w_o1_np = np.ascontiguousarray(W_o[:XD]).astype(F16)     # [64,128]
    w_o2_np = np.ascontiguousarray(W_o[XD:]).astype(F16)     # [128,128]

    x8t = np.ascontiguousarray(x.T).astype(F8E3)             # [64, N]
    ea8t = np.ascontiguousarray(edge_attr.T).astype(F8E3)    # [16, E]

    in_maps = []
    for c in range(NCORES):
        e0, e1 = core_starts[c], core_starts[c + 1]
        ld = lds[c]
        ne = e1 - e0
        eids = order[e0:e1]

        win = np.searchsorted(win_starts, ld, side="right") - 1
        first_e = cums[c][win_starts[win]]
        rank = np.arange(ne) - first_e
        if ne and rank.max() >= P:
            return _reference_fallback(x, edge_index, edge_attr, rev,
                                       W_i, b_i, W_h, b_h, W_o, b_o)
        slots = win * P + rank

        zt_np = np.zeros((80, NWT * P), dtype=F8E3)
        zt_np[0:XD, slots] = x8t[:, src[eids]]
        zt_np[XD:XD + EAD, slots] = ea8t[:, eids]

        s4_np = np.zeros((P, NSUP * SUP), dtype=F8E4)
        s4_np[rank, ld] = 1.0

        xct_np = np.zeros((XD, NPAD), dtype=F8E3)
        xct_np[:, :NL] = x8t[:, c * NL:(c + 1) * NL]

        in_maps.append({
            "zt": zt_np, "s4d": s4_np, "xct": xct_np,
            "w_ih": w_ih_np, "w_o1": w_o1_np, "w_o2": w_o2_np,
        })

    if win_key not in _PROGRAM_CACHE:
        _PROGRAM_CACHE.clear()
        _PROGRAM_CACHE[win_key] = _build_program(win_key)
    nc = _PROGRAM_CACHE[win_key]

    import os
    trace = bool(os.environ.get("BMP_TRACE"))
    res = run_bass_kernel_spmd(nc, in_maps, core_ids=list(range(NCORES)),
                               trace=trace)
    if trace:
        global LAST_EXEC_TIME_NS, LAST_TRACE
        LAST_EXEC_TIME_NS = res.exec_time_ns
        LAST_TRACE = res.instructions_and_trace
    out = np.empty((N, HID), dtype=F32)
    for c in range(NCORES):
        out[c * NL:(c + 1) * NL] = res.results[c]["outT"][:, :NL].T.astype(F32)
    return out


# revision 18
# speedup vs baseline: 1.1303x; 1.1303x over previous
"""Trainium2 Bass kernel for nn_BondMessagePassing (D-MPNN style GNN).

Contract: kernel(**inputs) takes FULL unsharded inputs (as produced by the
reference's setup_inputs) and returns the FULL output [400000, 128] float32.

Math: the reference builds edges in exact reverse pairs (edge 2k+1 is the
reverse of edge 2k, rev_edge_index = i^1), which makes dst[rev] == src.
Consequently the two scatter-adds inside every message-passing iteration
cancel exactly, so m == 0 through the loop and the output reduces to

    h   = relu([x[src], edge_attr] @ W_i)
    m   = scatter_add(h, dst)
    out = relu([x, m] @ W_o)

(biases are all zero for the documented generator; nonzero biases or a
broken reverse-pair identity fall back to an exact numpy replication).

Sharding: nodes are range-partitioned across the 8 cores (50000 nodes
each); each core receives exactly the edges whose dst lands in its range
(sorted by dst) so the scatter-add is core-local and the output rows are a
contiguous slice -- no collectives.

v2 design (vs the 180us baseline):
 - Variable-boundary windows: each 512-node supertile is split into <=5
   node ranges chosen on the host so that EVERY core has <=128 edges per
   range (edges are sorted by dst, so a window's edges are a consecutive
   run; slot = 128*window + rank). This removes the overflow tile
   entirely: the scatter-add streams exactly 512 one-hot columns per
   supertile (down from 1024) and the one-hot DMA drops from 104KB to
   64KB per supertile.
 - zt and xct are fp8e3m4 (4 mantissa bits): halves the edge-payload DMA
   vs fp16 at ~1e-3 cost in relative error (1e-2 total, gate is 2e-2).
 - PSUM drains are spread over three engines: scalar does the h-relu,
   vector does the m-cast, gpsimd (Pool) does the out-relu.
 - The PE stream is software-pipelined across supertiles:
   [h0(g+1), out1(g+1), scatter(g), out2(g-1)] so the PE never sits
   behind a drain it just requested.
"""

import ml_dtypes
import numpy as np

# ---- problem constants (hardcoded per contract) ----
N = 400000
E = 400000
XD = 64        # node feature dim
EAD = 16       # edge feature dim
HID = 128
DEPTH = 5
NCORES = 8
NL = N // NCORES          # nodes per core
SUP = 512                 # nodes per supertile (one PSUM bank of fp32)
NSUP = (NL + SUP - 1) // SUP
NPAD = NSUP * SUP
P = 128
MAXW = 6                  # max windows per supertile supported by the program

F16 = np.float16
F32 = np.float32
F8E4 = ml_dtypes.float8_e4m3
F8E3 = ml_dtypes.float8_e3m4


def _check_fast_path_ok(src, dst, rev, x, edge_attr, b_i, b_h, b_o):
    """True iff the loop-cancellation identity holds, biases are zero, and
    fp8/fp16 is safe."""
    if src.shape != (E,) or dst.shape != (E,) or rev.shape != (E,):
        return False
    if np.any(b_i) or np.any(b_h) or np.any(b_o):
        return False
    if rev.min() < 0 or rev.max() >= E:
        return False
    seen = np.zeros(E, dtype=bool)
    seen[rev] = True
    if not seen.all():
        return False
    if not np.array_equal(dst[rev], src):
        return False
    if src.min() < 0 or src.max() >= N or dst.min() < 0 or dst.max() >= N:
        return False
    mx = float(np.abs(x).max(initial=0.0))
    mea = float(np.abs(edge_attr).max(initial=0.0))
    if max(mx, mea) > 14.0:       # fp8e3m4 max is 15.5
        return False
    return True


def _reference_fallback(x, edge_index, edge_attr, rev_edge_index,
                        W_i, b_i, W_h, b_h, W_o, b_o):
    def san(t):
        return np.nan_to_num(t, nan=0.0, posinf=1000.0, neginf=-1000.0)

    src, dst = edge_index[0], edge_index[1]
    h0 = np.maximum(
        np.concatenate([x[src], edge_attr], axis=1) @ W_i + b_i, 0.0
    ).astype(F32)
    h = h0
    for _ in range(1, DEPTH):
        m = np.zeros_like(h)
        np.add.at(m, dst, h)
        np.add.at(m, src, -h[rev_edge_index])
        m = san(m) @ W_h + b_h
        h = np.maximum(h0 + m, 0.0).astype(F32)
    m_final = np.zeros_like(h)
    np.add.at(m_final, dst, h)
    h_cat = np.concatenate([x, san(m_final)], axis=1)
    out = np.maximum(h_cat @ W_o + b_o, 0.0).astype(F32)
    return san(out)


def _chunk_schedule():
    # even-sized chunks (supertiles are processed in pairs)
    sched = []
    t0 = 0
    for g in (2, 4):
        sched.append((t0, g)); t0 += g
    while NSUP - t0 > 6:
        sched.append((t0, 6)); t0 += 6
    for g in (6, 4, 2):
        if NSUP - t0 >= g:
            sched.append((t0, g)); t0 += g
    while t0 < NSUP:
        sched.append((t0, 2)); t0 += 2
    assert sum(g for _, g in sched) == NSUP
    assert all(g % 2 == 0 for _, g in sched)
    return sched


_PROGRAM_CACHE = {}


def _build_program(win_key):
    """Build the SPMD Bass program.

    win_key: tuple over supertiles of tuples of (offset, width) windows,
    offsets local to the supertile, widths summing to 512. Each window has
    <=128 edges on every core.
    """
    import concourse.bacc as bacc
    import concourse.mybir as mybir
    import concourse.tile as tile

    wins = [list(ws) for ws in win_key]
    nwl = [len(ws) for ws in wins]
    wib = np.concatenate(([0], np.cumsum(nwl)))   # window-id base per T
    NWT = int(wib[-1])

    nc = bacc.Bacc("TRN2", target_bir_lowering=False, debug=False,
                   num_devices=NCORES)
    dt = mybir.dt

    zt = nc.dram_tensor("zt", [80, NWT * P], dt.float8e3,
                        kind="ExternalInput")
    s4d = nc.dram_tensor("s4d", [P, NSUP * SUP], dt.float8e4,
                         kind="ExternalInput")
    xct = nc.dram_tensor("xct", [XD, NPAD], dt.float8e3,
                         kind="ExternalInput")
    w_ih = nc.dram_tensor("w_ih", [80, HID], dt.float16, kind="ExternalInput")
    w_o1 = nc.dram_tensor("w_o1", [XD, HID], dt.float16, kind="ExternalInput")
    w_o2 = nc.dram_tensor("w_o2", [HID, HID], dt.float16, kind="ExternalInput")
    # output produced TRANSPOSED ([hidden, node]); host transposes back
    outT = nc.dram_tensor("outT", [HID, NPAD], dt.float16,
                          kind="ExternalOutput")

    RELU = mybir.ActivationFunctionType.Relu
    sched = _chunk_schedule()
    nchunks = len(sched)
    chunk_of = np.empty(NSUP, dtype=np.int64)
    for j, (TT, Gc) in enumerate(sched):
        chunk_of[TT:TT + Gc] = j

    with tile.TileContext(nc) as tc:
        with (
            tc.tile_pool(name="consts", bufs=1) as consts,
            tc.tile_pool(name="ztp", bufs=3) as ztp,
            tc.tile_pool(name="sp", bufs=3) as sp,
            tc.tile_pool(name="xctp", bufs=3) as xctp,
            tc.tile_pool(name="hp", bufs=2) as hp,
            tc.tile_pool(name="mp", bufs=2) as mp,
            tc.tile_pool(name="op", bufs=3) as op,
            tc.tile_pool(name="hps", bufs=1, space="PSUM") as hps,
            tc.tile_pool(name="mps", bufs=1, space="PSUM") as mps,
            tc.tile_pool(name="ops", bufs=1, space="PSUM") as ops,
        ):
            zt_c, s4_c, xct_c, o_buf = {}, {}, {}, {}

            def issue_chunk(j):
                TT, Gc = sched[j]
                zc0, zc1 = int(wib[TT]) * P, int(wib[TT + Gc]) * P
                zt_t = ztp.tile([80, zc1 - zc0], dt.float8e3, tag="ztc",
                                name=f"zt_{j}")
                nc.sync.dma_start(out=zt_t, in_=zt[:, zc0:zc1])
                zt_c[j] = zt_t
                s4_t = sp.tile([P, Gc * SUP], dt.float8e4, tag="s4c",
                               name=f"s4_{j}")
                nc.sync.dma_start(
                    out=s4_t, in_=s4d[:, TT * SUP:(TT + Gc) * SUP])
                s4_c[j] = s4_t
                xct_t = xctp.tile([XD, Gc * SUP], dt.float8e3, tag="xctc",
                                  name=f"xct_{j}")
                nc.sync.dma_start(
                    out=xct_t, in_=xct[:, TT * SUP:(TT + Gc) * SUP])
                xct_c[j] = xct_t

            issue_chunk(0)
            w_ih_t = consts.tile([80, HID], dt.float16)
            nc.scalar.dma_start(out=w_ih_t, in_=w_ih[:, :])
            w_o1_t = consts.tile([XD, HID], dt.float16)
            nc.scalar.dma_start(out=w_o1_t, in_=w_o1[:, :])
            w_o2_t = consts.tile([HID, HID], dt.float16)
            nc.scalar.dma_start(out=w_o2_t, in_=w_o2[:, :])

            issue_chunk(1)

            h_sb_t, m_t_t, o_ps_t = {}, {}, {}
            NPAIR = NSUP // 2

            for i in range(NPAIR + 2):
                # --- A/C: h0 matmuls + one paired h-relu for pair i ---
                pA = i
                if pA < NPAIR:
                    g0 = 2 * pA
                    j = int(chunk_of[g0])
                    if g0 == sched[j][0] and j + 2 < nchunks and \
                            (j + 2) not in zt_c:
                        issue_chunk(j + 2)
                    TT, Gc = sched[j]
                    h_ps = hps.tile([P, 10 * HID], mybir.dt.float32,
                                    tag="hps", name=f"hps_{pA}")
                    nwin = [nwl[g0], nwl[g0 + 1]]
                    for s in (0, 1):
                        g = g0 + s
                        zoff = (int(wib[g]) - int(wib[TT])) * P
                        for w in range(nwin[s]):
                            c0 = (s * 5 + w) * HID
                            nc.tensor.matmul(
                                h_ps[:, c0:c0 + HID],
                                zt_c[j][:, zoff + w * P:zoff + (w + 1) * P],
                                w_ih_t, start=True, stop=True,
                                skip_group_check=True)
                    # one relu for the whole pair (two only in the rare case
                    # of a <5-window first supertile, to avoid reading the
                    # uninitialized PSUM gap)
                    h_sb = hp.tile([P, 10 * HID], dt.float16,
                                   tag="hsb", name=f"hsb_{pA}")
                    h_sb_t[pA] = h_sb
                    if nwin[0] == 5:
                        hi_col = (5 + nwin[1]) * HID
                        nc.scalar.activation(h_sb[:, :hi_col],
                                             h_ps[:, :hi_col], RELU)
                    else:
                        nc.scalar.activation(
                            h_sb[:, :nwin[0] * HID],
                            h_ps[:, :nwin[0] * HID], RELU)
                        nc.scalar.activation(
                            h_sb[:, 5 * HID:(5 + nwin[1]) * HID],
                            h_ps[:, 5 * HID:(5 + nwin[1]) * HID], RELU)

                # --- F/G: out2 + out relu halves for pair i-2 ---
                pF = i - 2
                if 0 <= pF < NPAIR:
                    g0 = 2 * pF
                    j = int(chunk_of[g0])
                    TT, Gc = sched[j]
                    if j not in o_buf:
                        o_buf[j] = op.tile([P, Gc * SUP], dt.float16,
                                           tag="obuf", name=f"ob_{j}")
                    o_ps = o_ps_t.pop(pF)
                    for s in (0, 1):
                        nc.tensor.matmul(o_ps[:, s * SUP:(s + 1) * SUP],
                                         w_o2_t,
                                         m_t_t[pF][:, s * SUP:(s + 1) * SUP],
                                         start=False, stop=True,
                                         skip_group_check=True)
                    del m_t_t[pF]
                    gl = g0 - TT
                    ob = o_buf[j][:, gl * SUP:(gl + 2) * SUP]
                    if pF % 2 == 0:
                        nc.scalar.activation(ob, o_ps, RELU)
                    else:
                        nc.vector.tensor_scalar_max(ob, o_ps, 0.0)
                    if g0 + 2 == TT + Gc:
                        # chunk finished: ship it from the scalar DGE queue
                        nc.scalar.dma_start(
                            out=outT[:, TT * SUP:(TT + Gc) * SUP],
                            in_=o_buf[j])

                # --- D/E: scatter + paired m cast for pair i-1 ---
                pD = i - 1
                if 0 <= pD < NPAIR:
                    g0 = 2 * pD
                    j = int(chunk_of[g0])
                    TT, Gc = sched[j]
                    m_ps = mps.tile([P, 2 * SUP], mybir.dt.float32,
                                    tag="mps", name=f"mps_{pD}")
                    h_sb = h_sb_t.pop(pD)
                    for s in (0, 1):
                        g = g0 + s
                        gl = g - TT
                        for w, (off, width) in enumerate(wins[g]):
                            nc.tensor.matmul(
                                m_ps[:, s * SUP + off:s * SUP + off + width],
                                h_sb[:, (s * 5 + w) * HID:
                                     (s * 5 + w + 1) * HID],
                                s4_c[j][:, gl * SUP + off:
                                        gl * SUP + off + width],
                                start=True, stop=True, skip_group_check=True)
                    m_t = mp.tile([P, 2 * SUP], dt.float16, tag="mt",
                                  name=f"mt_{pD}")
                    m_t_t[pD] = m_t
                    nc.vector.tensor_copy(m_t, m_ps)

                # --- B: out1 for pair i (last: single-buffer o_ps reuse) ---
                if pA < NPAIR:
                    g0 = 2 * pA
                    j = int(chunk_of[g0])
                    TT, Gc = sched[j]
                    o_ps = ops.tile([P, 2 * SUP], mybir.dt.float32,
                                    tag="ops", name=f"ops_{pA}")
                    o_ps_t[pA] = o_ps
                    for s in (0, 1):
                        g = g0 + s
                        gl = g - TT
                        nc.tensor.matmul(o_ps[:, s * SUP:(s + 1) * SUP],
                                         w_o1_t,
                                         xct_c[j][:, gl * SUP:(gl + 1) * SUP],
                                         start=True, stop=False,
                                         skip_group_check=True)

    nc.compile()
    return nc


def kernel(**inputs):
    x = np.ascontiguousarray(np.asarray(inputs["x"]), dtype=F32)
    edge_index = np.asarray(inputs["edge_index"]).astype(np.int64)
    edge_attr = np.ascontiguousarray(np.asarray(inputs["edge_attr"]), dtype=F32)
    rev = np.asarray(inputs["rev_edge_index"]).astype(np.int64)
    W_i = np.asarray(inputs["W_i"], dtype=F32)
    b_i = np.asarray(inputs["b_i"], dtype=F32)
    W_h = np.asarray(inputs["W_h"], dtype=F32)
    b_h = np.asarray(inputs["b_h"], dtype=F32)
    W_o = np.asarray(inputs["W_o"], dtype=F32)
    b_o = np.asarray(inputs["b_o"], dtype=F32)

    src, dst = edge_index[0], edge_index[1]

    if not _check_fast_path_ok(src, dst, rev, x, edge_attr, b_i, b_h, b_o):
        return _reference_fallback(x, edge_index, edge_attr, rev,
                                   W_i, b_i, W_h, b_h, W_o, b_o)

    from concourse.bass_utils import run_bass_kernel_spmd

    # ---- host-side graph partition / sort (indices only) ----
    order = np.argsort(dst, kind="stable")
    dst_s = dst[order]
    core_starts = np.searchsorted(dst_s, np.arange(0, N + NL, NL))

    lds = []                       # per-core sorted local dst
    cums = np.zeros((NCORES, NPAD + 1), dtype=np.int64)
    for c in range(NCORES):
        e0, e1 = core_starts[c], core_starts[c + 1]
        ld = dst_s[e0:e1] - c * NL
        lds.append(ld)
        cnt = np.bincount(ld, minlength=NPAD)
        cums[c, 1:] = np.cumsum(cnt)

    # ---- joint greedy window boundaries (shared across all 8 cores) ----
    wins = []                      # per T: list of (offset, width)
    win_starts = []                # flat window start node ids (local)
    ok = True
    for T in range(NSUP):
        lo, hi = T * SUP, (T + 1) * SUP
        b = lo
        ws = []
        while b < hi:
            b2 = hi
            for c in range(NCORES):
                t = cums[c, b] + P
                bb = int(np.searchsorted(cums[c], t, side="right")) - 1
                b2 = min(b2, bb)
            if b2 <= b:
                ok = False
                break
            ws.append((b - lo, b2 - b))
            win_starts.append(b)
            b = b2
        if not ok or len(ws) > MAXW:
            ok = False
            break
        wins.append(tuple(ws))
    if not ok:
        return _reference_fallback(x, edge_index, edge_attr, rev,
                                   W_i, b_i, W_h, b_h, W_o, b_o)

    win_starts = np.asarray(win_starts, dtype=np.int64)
    NWT = len(win_starts)
    win_key = tuple(wins)

    # ---- shared constant tensors ----
    w_ih_np = np.ascontiguousarray(W_i).astype(F16)          # [80,128]
    w_o1_np = np.ascontiguousarray(W_o[:XD]).astype(F16)     # [64,128]
    w_o2_np = np.ascontiguousarray(W_o[XD:]).astype(F16)     # [128,128]

    x8t = np.ascontiguousarray(x.T).astype(F8E3)             # [64, N]
    ea8t = np.ascontiguousarray(edge_attr.T).astype(F8E3)    # [16, E]

    in_maps = []
    for c in range(NCORES):
        e0, e1 = core_starts[c], core_starts[c + 1]
        ld = lds[c]
        ne = e1 - e0
        eids = order[e0:e1]

        win = np.searchsorted(win_starts, ld, side="right") - 1
        first_e = cums[c][win_starts[win]]
        rank = np.arange(ne) - first_e
        if ne and rank.max() >= P:
            return _reference_fallback(x, edge_index, edge_attr, rev,
                                       W_i, b_i, W_h, b_h, W_o, b_o)
        slots = win * P + rank

        zt_np = np.zeros((80, NWT * P), dtype=F8E3)
        zt_np[0:XD, slots] = x8t[:, src[eids]]
        zt_np[XD:XD + EAD, slots] = ea8t[:, eids]

        s4_np = np.zeros((P, NSUP * SUP), dtype=F8E4)
        s4_np[rank, ld] = 1.0

        xct_np = np.zeros((XD, NPAD), dtype=F8E3)
        xct_np[:, :NL] = x8t[:, c * NL:(c + 1) * NL]

        in_maps.append({
            "zt": zt_np, "s4d": s4_np, "xct": xct_np,
            "w_ih": w_ih_np, "w_o1": w_o1_np, "w_o2": w_o2_np,
        })

    if win_key not in _PROGRAM_CACHE:
        _PROGRAM_CACHE.clear()
        _PROGRAM_CACHE[win_key] = _build_program(win_key)
    nc = _PROGRAM_CACHE[win_key]

    import os
    trace = bool(os.environ.get("BMP_TRACE"))
    res = run_bass_kernel_spmd(nc, in_maps, core_ids=list(range(NCORES)),
                               trace=trace)
    if trace:
        global LAST_EXEC_TIME_NS, LAST_TRACE
        LAST_EXEC_TIME_NS = res.exec_time_ns
        LAST_TRACE = res.instructions_and_trace
    out = np.empty((N, HID), dtype=F32)
    for c in range(NCORES):
        out[c * NL:(c + 1) * NL] = res.results[c]["outT"][:, :NL].T.astype(F32)
    return out
